# revision 13
# baseline (speedup 1.0000x reference)
"""Single-query attention ("context inner product") on 8 trn2 NeuronCores.

    scores  = enc @ dec[0]          enc: [S=16384, H=2048] f32, dec: [1, H]
    weights = softmax(scores)
    context = weights @ enc         -> [1, H]

Sharding: enc split along seq_len across 8 cores (2048 rows each); each core
streams its 16 MB shard once (memory-bound) and emits an unnormalized partial
context plus per-block weight sums; the host does the final divide.

v3 layout (vs the ~60-69us v2):
  - The whole enc stream is plain-f32 HWDGE DMA on the sync queue: no SWDGE
    Q7 descriptor-generation serialization at kernel start (stream begins
    ~1.5us earlier) and no inline cast. dec rides SWDGE slot 2 is gone; dec
    is HWDGE too and broadcast across partitions by gpsimd
    partition_broadcast (no PE outer product, no PSUM).
  - Scores: fused mul+rowsum (scalar_tensor_tensor, f32) alternating
    between DVE (even units) and Pool (odd units), both idle-heavy now.
    Dump tiles are written fp16 to halve SBUF traffic/footprint.
  - Context matmuls run in float32r (1 cycle/row at >=256 moving cols, the
    same rate as fp16) straight off the f32 tiles via AP bitcast.
  - PE p-state: the tensor engine only reaches 2.4 GHz after ~3us of
    CONTINUOUS execution; gated per-unit bursts run at 1.2 GHz (and 0.65
    after idles). Paced filler matmuls (fp16 ones x ones into a scratch
    PSUM bank) keep PE busy from the broadcast to the last context matmul.
  - Tail: last tile lands as 4 column chunks; chunk partials on Pool while
    DVE takes the final chunk; PSUM banks drain on DVE/Pool/ACT in
    parallel; two output DMAs (sync: banks 0-1, scalar: banks 2-3 + norm).
"""

import numpy as np

S, H = 16384, 2048
N_CORES = 8
S_LOCAL = S // N_CORES  # 2048
P = 128                 # SBUF partitions
HB = 512                # f32 elements per PSUM bank
N_BANKS = H // HB       # 4
N_UNITS = S_LOCAL // P  # 16 score units of 128 rows

# tile row counts: first small for fast start, last three 128-row so the
# final units gate on fresh arrivals and the tail chain is short
TILE_ROWS = [128, 256, 256, 256, 256, 256, 256, 128, 128, 128]

_CACHE: dict = {}


def _build():
    import concourse.bacc as bacc
    import concourse.tile as tile
    from concourse import mybir

    f32 = mybir.dt.float32
    f32r = mybir.dt.float32r
    f16 = mybir.dt.float16
    nc = bacc.Bacc(
        "TRN2", target_bir_lowering=False, debug=False, num_devices=N_CORES
    )
    # flat DRAM view so multi-row-per-partition DMA tiles are plain linear
    enc = nc.dram_tensor("enc", [S_LOCAL * H], f32, kind="ExternalInput").ap()
    dec = nc.dram_tensor("dec", [1, H], f32, kind="ExternalInput").ap()
    # out row: [0:H] unnormalized context, [H:H+16] per-unit weight sums
    ctx_d = nc.dram_tensor("ctx", [1, H + N_UNITS], f32, kind="ExternalOutput").ap()

    # --- stream timing model (ms) for Tile scheduler hints -----------------
    T0 = 0.0079            # first enc byte lands
    MS_PER_MB = 0.00243    # per MiB of f32 source
    arrive = []
    cum = 0.0
    for r in TILE_ROWS:
        cum += r * H * 4 / 2**20
        arrive.append(T0 + MS_PER_MB * cum)
    # per-unit arrival (units in tile order; 256-row tiles carry 2 units)
    unit_arrive = []
    units = []  # (tile index, col offset)
    for i, r in enumerate(TILE_ROWS):
        for h in range(r // P):
            units.append((i, h * H))
            unit_arrive.append(arrive[i])

    with tile.TileContext(nc) as tc:
        with (
            tc.tile_pool(name="singles", bufs=1) as singles,
            tc.tile_pool(name="dump_v", bufs=2) as dump_v,
            tc.tile_pool(name="dump_p", bufs=2) as dump_p,
            tc.tile_pool(name="dump_s", bufs=2) as dump_s,
            tc.tile_pool(name="sc_pool", bufs=4) as sc_pool,
            tc.tile_pool(name="psum", bufs=1, space="PSUM") as psum_pool,
        ):
            dec32 = singles.tile([1, H], f32)
            dec_b = singles.tile([P, H], f32)
            ones = singles.tile([P, 1], f32r)
            ones_row16 = singles.tile([1, P], f16)
            ones512 = singles.tile([1, HB], f16)
            w_all = singles.tile([P, N_UNITS], f32r)
            out_sb = singles.tile([1, H + N_UNITS], f32)
            # enc tiles are float32r (same bits as f32): the PE consumes them
            # directly in fp32r matmuls (1 cycle/row) and the BIR verifier
            # wants fp32r operands produced as fp32r; DVE/Pool read them
            # through a f32 bitcast view.
            enc_t = [
                singles.tile([P, r * (H // P)], f32r, name=f"enc_t{i}")
                for i, r in enumerate(TILE_ROWS)
            ]

            enc2d = enc.rearrange("(s h) -> s h", h=H)
            last = len(TILE_ROWS) - 1
            row0 = 0
            dma_rows = []
            for r in TILE_ROWS:
                dma_rows.append(row0)
                row0 += r

            # --- enc + dec loads --------------------------------------------
            # HWDGE descriptor generation caps out near ~218 GB/s with 8 KB
            # descriptors (measured), while SWDGE pre-generates its ring from
            # Q7 and sustains ~412 GB/s. So: tile0 + dec ride HWDGE (sync
            # queue, starts ~1us before Q7 gets going), the bulk of the
            # stream is SWDGE on the gpsimd queue. f32r both sides = plain
            # copy, no cast anywhere.
            nc.sync.dma_start(
                out=enc_t[0][:],
                in_=enc[dma_rows[0] * H : (dma_rows[0] + TILE_ROWS[0]) * H].bitcast(f32r),
            )
            nc.sync.dma_start(out=dec32[:], in_=dec[:])
            trig_hint = [
                0.0066, 0.0090, 0.0115, 0.0150, 0.0200, 0.0245, 0.0290, 0.0340,
            ]
            for i in range(1, last):
                with tc.tile_wait_until(trig_hint[i - 1]):
                    nc.gpsimd.dma_start(
                        out=enc_t[i][:],
                        in_=enc[dma_rows[i] * H : (dma_rows[i] + TILE_ROWS[i]) * H].bitcast(f32r),
                    )
            # last tile arrives as 4 column chunks so its score partials can
            # start before the final bytes land (shortens the tail chain)
            r0 = dma_rows[last]
            for c in range(4):
                with tc.tile_wait_until(0.0370 + 0.0008 * c):
                    nc.gpsimd.dma_start(
                        out=enc_t[last][:, c * HB : (c + 1) * HB],
                        in_=enc2d[r0 : r0 + P, c * HB : (c + 1) * HB].bitcast(f32r),
                    )

            nc.vector.memset(ones_row16[:], 1.0)
            nc.vector.memset(ones512[:], 1.0)
            # memset can't write f32r; round through an ACT copy instead
            ones_f = singles.tile([P, 1], f32)
            nc.vector.memset(ones_f[:], 1.0)
            nc.scalar.activation(
                out=ones[:], in_=ones_f[:],
                func=mybir.ActivationFunctionType.Copy,
            )

            # dec broadcast across partitions on Pool (gpsimd)
            with tc.tile_wait_until(T0 + 0.0008):
                nc.gpsimd.partition_broadcast(dec_b[:], dec32[:])

            ctx_psum = [
                psum_pool.tile([1, HB], f32, tag=f"ctxb{j}", name=f"ctxb{j}")
                for j in range(N_BANKS)
            ]
            norm_psum = psum_pool.tile([1, N_UNITS], f32, tag="normp")
            warm_psum = psum_pool.tile([P, HB], f32, tag="warm")

            sc_of = {}
            for k in range(len(units)):
                sc_of[k] = sc_pool.tile([P, 1], f32, tag="sc", name="sc", bufs=4)

            def filler(cols):
                nc.tensor.matmul(
                    warm_psum[:, :cols],
                    ones_row16[:],
                    ones512[:, :cols],
                    start=True,
                    stop=True,
                )

            def score_fused(eng, e, sc, dump_pool, cols=H, dcol=0):
                # fused mul+rowsum in one DVE pass (stt is DVE-only ISA)
                prod = dump_pool.tile([P, H], f16, tag="prod", name="prod")
                eng.scalar_tensor_tensor(
                    out=prod[:, dcol : dcol + cols],
                    in0=e,
                    scalar=1.0,
                    in1=dec_b[:, dcol : dcol + cols],
                    op0=mybir.AluOpType.mult,
                    op1=mybir.AluOpType.mult,
                    accum_out=sc[:],
                )

            def score_split(e, sc, cols=H, dcol=0):
                # Pool does the elementwise mul (fp16 dump), ACT the rowsum
                prod = dump_p.tile([P, H], f16, tag="prod", name="prod")
                nc.gpsimd.tensor_tensor(
                    out=prod[:, dcol : dcol + cols],
                    in0=e,
                    in1=dec_b[:, dcol : dcol + cols],
                    op=mybir.AluOpType.mult,
                )
                dump = dump_s.tile([P, H], f16, tag="dumps", name="dumps")
                nc.scalar.activation(
                    out=dump[:, dcol : dcol + cols],
                    in_=prod[:, dcol : dcol + cols],
                    func=mybir.ActivationFunctionType.Copy,
                    accum_out=sc[:],
                )

            def ctx_matmuls(k, e, first, last_u):
                nc.scalar.activation(
                    out=w_all[:, k : k + 1],
                    in_=sc_of[k][:],
                    func=mybir.ActivationFunctionType.Exp,
                )
                for b in range(N_BANKS):
                    nc.tensor.matmul(
                        ctx_psum[b][:],
                        w_all[:, k : k + 1],
                        e[:, b * HB : (b + 1) * HB],
                        start=first,
                        stop=last_u,
                    )

            # --- PE p-state fillers: the tensor engine needs ~3us of
            # CONTINUOUS execution to reach 2.4 GHz; idle gaps drop it to
            # 1.2/0.65 GHz. Mid-stream that's harmless (PE has slack), but
            # the tail matmuls are latency-critical, so keep PE busy without
            # gaps from ~T-6.5us through the last context matmul.
            n_units = len(units)

            HQ = HB // 2  # 256
            for k, (ti, off) in enumerate(units):
                e = enc_t[ti][:, off : off + H]
                dve = k % 2 == 0
                eng = nc.vector if dve else nc.gpsimd
                dpool = dump_v if dve else dump_p
                e32 = e.bitcast(f32)
                dve = k % 2 == 0
                if k == n_units - 1:
                    # chunked last unit: 3 column-chunk partials on Pool as
                    # the chunks land, final chunk + combine on DVE
                    scp = [
                        sc_pool.tile([P, 1], f32, tag=f"scp{c}", name=f"scp{c}")
                        for c in range(3)
                    ]
                    t_base = arrive[ti - 1]
                    t_step = (arrive[ti] - t_base) / 4
                    for c in range(3):
                        with tc.tile_wait_until(t_base + (c + 1) * t_step):
                            score_split(
                                e32[:, c * HB : (c + 1) * HB],
                                scp[c], cols=HB, dcol=c * HB,
                            )
                        if c == 1:
                            with tc.tile_wait_until(t_base + 2 * t_step + 0.0006):
                                nc.gpsimd.tensor_add(scp[0][:], scp[0][:], scp[1][:])
                        if c == 2:
                            with tc.tile_wait_until(t_base + 3 * t_step + 0.0006):
                                nc.gpsimd.tensor_add(scp[0][:], scp[0][:], scp[2][:])
                    with tc.tile_wait_until(arrive[ti]):
                        scl = sc_pool.tile([P, 1], f32, tag="scl", name="scl")
                        score_fused(
                            nc.vector, e32[:, 3 * HB :], scl, dump_v,
                            cols=HB, dcol=3 * HB,
                        )
                        nc.vector.tensor_add(sc_of[k][:], scp[0][:], scl[:])
                        ctx_matmuls(k, e, k == 0, True)
                    continue
                with tc.tile_wait_until(unit_arrive[k]):
                    if dve:
                        score_fused(nc.vector, e32, sc_of[k], dump_v)
                    else:
                        score_split(e32, sc_of[k])
                    ctx_matmuls(k, e, k == 0, False)
                # tail p-state bursts: after u12's matmuls, ramp PE and keep
                # it busy until u13's matmuls are ready; after u13's, bridge
                # to u14's (u15's chain directly behind u14's, no gap).
                if k == n_units - 4:
                    for j in range(13):
                        with tc.tile_wait_until(arrive[-4] + 0.0026 + 0.00013 * j):
                            filler(HB)
                elif k == n_units - 3:
                    for j in range(8):
                        with tc.tile_wait_until(arrive[-3] + 0.0032 + 0.00013 * j):
                            filler(HB)

            # softmax normalizer: [1,16] row of per-unit weight sums, folded
            # into the context output row (no separate norm DMA)
            nc.tensor.matmul(
                norm_psum[:], ones[:], w_all[:], start=True, stop=True
            )
            # PSUM bank drains alternate DVE / ACT (gpsimd cannot read PSUM)
            nc.vector.tensor_copy(out_sb[:, 0:HB], ctx_psum[0][:])
            nc.scalar.copy(out_sb[:, HB : 2 * HB], ctx_psum[1][:])
            nc.vector.tensor_copy(out_sb[:, 2 * HB : 3 * HB], ctx_psum[2][:])
            nc.scalar.copy(out_sb[:, 3 * HB : 4 * HB], ctx_psum[3][:])
            nc.vector.tensor_copy(out_sb[:, H : H + N_UNITS], norm_psum[:])
            # two output DMAs on separate HWDGE queues, each fired as soon as
            # its pair of bank copies lands
            nc.sync.dma_start(out=ctx_d[:, 0 : 2 * HB], in_=out_sb[:, 0 : 2 * HB])
            nc.scalar.dma_start(out=ctx_d[:, 2 * HB :], in_=out_sb[:, 2 * HB :])

    nc.compile()
    return nc


def _make_runner(nc):
    """Cached equivalent of bass2jax.run_bass_via_pjrt's multi-core path:
    build the sharded jitted executable once so warm calls skip re-tracing."""
    import jax
    import numpy as np
    from jax.experimental.shard_map import shard_map
    from jax.sharding import Mesh, PartitionSpec

    from concourse import bass2jax, mybir

    bass2jax.install_neuronx_cc_hook()
    assert nc.dbg_addr is None
    partition_name = nc.partition_id_tensor.name if nc.partition_id_tensor else None

    in_names, out_names, out_avals = [], [], []
    for alloc in nc.m.functions[0].allocations:
        if not isinstance(alloc, mybir.MemoryLocationSet):
            continue
        name = alloc.memorylocations[0].name
        if alloc.kind == "ExternalInput":
            if name != partition_name:
                in_names.append(name)
        elif alloc.kind == "ExternalOutput":
            out_names.append(name)
            out_avals.append(
                jax.core.ShapedArray(
                    tuple(alloc.tensor_shape), mybir.dt.np(alloc.dtype)
                )
            )
    n_params = len(in_names)
    all_in = list(in_names) + list(out_names)
    if partition_name is not None:
        all_in.append(partition_name)
    donate = tuple(range(n_params, n_params + len(out_names)))

    def _body(*args):
        operands = list(args)
        if partition_name is not None:
            operands.append(bass2jax.partition_id_tensor())
        return tuple(
            bass2jax._bass_exec_p.bind(
                *operands,
                out_avals=tuple(out_avals),
                in_names=tuple(all_in),
                out_names=tuple(out_names),
                lowering_input_output_aliases=(),
                sim_require_finite=True,
                sim_require_nnan=True,
                nc=nc,
            )
        )

    devices = jax.devices()[:N_CORES]
    mesh = Mesh(np.asarray(devices), ("core",))
    nio = n_params + len(out_names)
    sharded = jax.jit(
        shard_map(
            _body,
            mesh=mesh,
            in_specs=(PartitionSpec("core"),) * nio,
            out_specs=(PartitionSpec("core"),) * len(out_names),
            check_rep=False,
        ),
        donate_argnums=donate,
        keep_unused=True,
    )

    def run(in_maps):
        concat_in = [
            np.concatenate([m[name] for m in in_maps], axis=0) for name in in_names
        ]
        concat_zeros = [
            np.zeros((N_CORES * a.shape[0], *a.shape[1:]), a.dtype)
            for a in out_avals
        ]
        out_arrs = sharded(*concat_in, *concat_zeros)
        return [
            {
                name: np.asarray(out_arrs[i]).reshape(
                    N_CORES, *out_avals[i].shape
                )[c]
                for i, name in enumerate(out_names)
            }
            for c in range(N_CORES)
        ]

    return run


def _run(encoder_hiddens, decoder_hidden, trace=False, **kw):
    from concourse.bass_utils import run_bass_kernel_spmd

    key = "nc_v3"
    if key not in _CACHE:
        _CACHE[key] = _build()
    nc = _CACHE[key]

    enc = np.ascontiguousarray(encoder_hiddens, dtype=np.float32)
    dec = np.ascontiguousarray(decoder_hidden, dtype=np.float32)
    in_maps = [
        {
            "enc": enc[c * S_LOCAL : (c + 1) * S_LOCAL].reshape(-1),
            "dec": dec,
        }
        for c in range(N_CORES)
    ]
    if trace:
        res = run_bass_kernel_spmd(
            nc, in_maps, core_ids=list(range(N_CORES)), trace=True, **kw
        )
        results = res.results
    else:
        rkey = "runner_v3"
        if rkey not in _CACHE:
            _CACHE[rkey] = _make_runner(nc)
        results = _CACHE[rkey](in_maps)
        res = None

    ctx = np.zeros((1, H), np.float64)
    z = 0.0
    for r in results:
        full = r["ctx"].astype(np.float64)
        ctx += full[:, :H]
        z += float(full[0, H:].sum())
    return (ctx / z).astype(np.float32), res


def kernel(encoder_hiddens, decoder_hidden):
    out, _ = _run(encoder_hiddens, decoder_hidden)
    return out


# revision 15
# speedup vs baseline: 1.9284x; 1.9284x over previous
"""Single-query attention ("context inner product") on 8 trn2 NeuronCores.

    scores  = enc @ dec[0]          enc: [S=16384, H=2048] f32, dec: [1, H]
    weights = softmax(scores)
    context = weights @ enc         -> [1, H]

Sharding: enc split along seq_len across 8 cores (2048 rows each); each core
streams its 16 MB shard once (memory-bound) and emits an unnormalized partial
context plus per-block weight sums; the host does the final divide.

v2 layout (vs the 69.7us baseline):
  - DMA: 10 big tiles; mid tiles pack 2 DRAM rows per partition line
    ([128, 4096] fp16 <- 256 contiguous rows) so each SWDGE packet reads
    16 KB instead of 8 KB. f32 -> fp16 cast inline. First/last tiles are
    128 rows so compute starts early and the tail chain is short.
  - dec is cast fp16 by its DMA (no 2us ACT cast on the critical path),
    then broadcast across partitions via a PE outer product.
  - scores: per 128-row unit, fused mul+rowsum on DVE (stt) for every
    third unit, else mul on DVE (fp16 2x) + rowsum on ACT — keeps both
    engines ~28us busy inside the ~41us DMA window so there is no backlog
    when the stream ends.
  - weights: exp on ACT into one W_all [128, 16] tile; the softmax
    normalizer comes from a single final PE matmul W_all^T @ ones -> [16,1]
    (replaces 16 per-block norm matmuls).
  - context: 4 PSUM-bank matmuls per unit, fp16, f32 PSUM accumulation
    across all units; PSUM drained by copies on vector/scalar/gpsimd in
    parallel, then two sync-engine DMAs (ctx row + norm partials).
"""

import numpy as np

S, H = 16384, 2048
N_CORES = 8
S_LOCAL = S // N_CORES  # 2048
P = 128                 # SBUF partitions
HB = 512                # f32 elements per PSUM bank
N_BANKS = H // HB       # 4
N_UNITS = S_LOCAL // P  # 16 score units of 128 rows

# tile row counts: first/last small for fast start + short tail
TILE_ROWS = [128, 256, 256, 256, 256, 256, 256, 128, 128, 128]

_CACHE: dict = {}


def _build(mm_dtype="f16"):
    import concourse.bacc as bacc
    import concourse.tile as tile
    from concourse import mybir

    f32 = mybir.dt.float32
    cdt = {"bf16": mybir.dt.bfloat16, "f16": mybir.dt.float16}[mm_dtype]
    nc = bacc.Bacc(
        "TRN2", target_bir_lowering=False, debug=False, num_devices=N_CORES
    )
    # flat DRAM view so multi-row-per-partition DMA tiles are plain linear
    enc = nc.dram_tensor("enc", [S_LOCAL * H], f32, kind="ExternalInput").ap()
    dec = nc.dram_tensor("dec", [1, H], f32, kind="ExternalInput").ap()
    # out row: [0:H] unnormalized context, [H:H+16] per-unit weight sums
    ctx_d = nc.dram_tensor("ctx", [1, H + N_UNITS], f32, kind="ExternalOutput").ap()

    with tile.TileContext(nc) as tc:
        with (
            tc.tile_pool(name="singles", bufs=1) as singles,
            tc.tile_pool(name="prod_pool", bufs=3) as prod_pool,
            tc.tile_pool(name="dump_pool", bufs=2) as dump_pool,
            tc.tile_pool(name="sc_pool", bufs=4) as sc_pool,
            tc.tile_pool(name="psum", bufs=1, space="PSUM") as psum_pool,
            tc.tile_pool(name="psum2", bufs=2, space="PSUM") as psum2_pool,
        ):
            dec16 = singles.tile([1, H], cdt)
            ones_row = singles.tile([1, P], cdt)
            ones512 = singles.tile([1, HB], cdt)
            ones = singles.tile([P, 1], cdt)
            dec_b = singles.tile([P, H], cdt)
            w_all = singles.tile([P, N_UNITS], cdt)
            out_sb = singles.tile([1, H + N_UNITS], f32)
            enc_t = [
                singles.tile([P, r * (H // P)], cdt, name=f"enc_t{i}")
                for i, r in enumerate(TILE_ROWS)
            ]

            # enc tile 0 first so the HBM stream starts immediately; the tiny
            # dec load slots in right after and its cast rides the DMA.
            row0 = 0
            dma_rows = []
            for i, r in enumerate(TILE_ROWS):
                dma_rows.append(row0)
                row0 += r
            enc2d = enc.rearrange("(s h) -> s h", h=H)
            last = len(TILE_ROWS) - 1
            # enc tile0's trigger first so the HBM stream starts as early as
            # possible; the tiny dec load rides second (its broadcast chain
            # has ~2us of slack before unit 0's scores need it).
            nc.gpsimd.dma_start(
                out=enc_t[0][:], in_=enc[dma_rows[0] * H : (dma_rows[0] + TILE_ROWS[0]) * H]
            )
            nc.gpsimd.dma_start(out=dec16[:], in_=dec[:])
            for i in range(1, last):
                nc.gpsimd.dma_start(
                    out=enc_t[i][:],
                    in_=enc[dma_rows[i] * H : (dma_rows[i] + TILE_ROWS[i]) * H],
                )
            # last tile arrives as 4 column chunks so its score partials can
            # start before the final bytes land (shortens the tail chain)
            r0 = dma_rows[last]
            for c in range(4):
                nc.gpsimd.dma_start(
                    out=enc_t[last][:, c * HB : (c + 1) * HB],
                    in_=enc2d[r0 : r0 + P, c * HB : (c + 1) * HB],
                )

            nc.vector.memset(ones_row[:], 1.0)
            nc.vector.memset(ones512[:], 1.0)
            nc.vector.memset(ones[:], 1.0)

            # Broadcast dec across partitions with a PE outer product.
            for b in range(N_BANKS):
                bc = psum2_pool.tile([P, HB], f32, tag="bc", name="bc")
                nc.tensor.matmul(
                    bc[:],
                    ones_row[:],
                    dec16[:, b * HB : (b + 1) * HB],
                    start=True,
                    stop=True,
                )
                eng = nc.vector.tensor_copy if b % 2 == 0 else nc.scalar.copy
                eng(dec_b[:, b * HB : (b + 1) * HB], bc[:])

            ctx_psum = [
                psum_pool.tile([1, HB], f32, tag=f"ctxb{j}", name=f"ctxb{j}")
                for j in range(N_BANKS)
            ]
            norm_psum = psum_pool.tile([1, N_UNITS], f32, tag="normp")
            warm_psum = psum_pool.tile([P, HB], f32, tag="warm")

            # (tile index, column offset of this 128-row unit within the tile)
            units = []
            for i, r in enumerate(TILE_ROWS):
                for h in range(r // P):
                    units.append((i, h * H))

            # fused stt on DVE for these units; the rest run mul(DVE)+rowsum(ACT).
            # Near the stream end u13/u14 run fused on DVE while u15's first
            # three column chunks ride Pool(mul)+ACT(rowsum).
            fused = {0, 3, 6, 9, 12, 13, 14}

            def score_fused(e, sc, eng=None):
                prod = prod_pool.tile([P, H], cdt, tag="prod", name="prod")
                (eng or nc.vector).scalar_tensor_tensor(
                    out=prod[:],
                    in0=e,
                    scalar=1.0,
                    in1=dec_b[:],
                    op0=mybir.AluOpType.mult,
                    op1=mybir.AluOpType.mult,
                    accum_out=sc[:],
                )

            def score_split(e, sc):
                prod = prod_pool.tile([P, H], cdt, tag="prod", name="prod")
                nc.vector.tensor_mul(prod[:], e, dec_b[:])
                dump = dump_pool.tile([P, H], cdt, tag="dump", name="dump")
                nc.scalar.activation(
                    out=dump[:],
                    in_=prod[:],
                    func=mybir.ActivationFunctionType.Copy,
                    accum_out=sc[:],
                )

            def ctx_matmuls(k, e, first, last_u):
                nc.scalar.activation(
                    out=w_all[:, k : k + 1],
                    in_=sc_of[k][:],
                    func=mybir.ActivationFunctionType.Exp,
                )
                for b in range(N_BANKS):
                    nc.tensor.matmul(
                        ctx_psum[b][:],
                        w_all[:, k : k + 1],
                        e[:, b * HB : (b + 1) * HB],
                        start=first,
                        stop=last_u,
                    )

            sc_of = {}
            n_units = len(units)
            for k, (ti, off) in enumerate(units):
                sc_of[k] = sc_pool.tile([P, 1], f32, tag="sc", name="sc", bufs=4)

            # Scheduler hint: real DMA arrival time (ms) for each tile's data.
            # The tile scheduler's own DMA model mis-orders the stream (it had
            # the last tile's chunks sequenced before ready mid-stream units),
            # so feed it the queue-order completion times instead.
            ms_per_mb = 0.00243
            arrive = []
            cum = 0.0
            for r in TILE_ROWS:
                cum += r * H * 4 / 2**20
                arrive.append(0.008 + ms_per_mb * cum)

            for k, (ti, off) in enumerate(units):
                e = enc_t[ti][:, off : off + H]
                if k == n_units - 1:
                    # chunked last unit: chunks 0-2 ride Pool(mul)+ACT(rowsum)
                    # as they land (DVE is busy with u14's fused pass); DVE
                    # takes the final chunk the moment the last bytes arrive.
                    scp = [
                        sc_pool.tile([P, 1], f32, tag=f"scp{c}", name=f"scp{c}")
                        for c in range(3)
                    ]
                    t_base = arrive[ti - 1] if ti else 0.008
                    t_step = (arrive[ti] - t_base) / 4
                    for c in range(3):
                        with tc.tile_wait_until(t_base + (c + 1) * t_step):
                            prodc = prod_pool.tile(
                                [P, HB], cdt, tag="prodc", name="prodc", bufs=2
                            )
                            nc.gpsimd.tensor_tensor(
                                out=prodc[:],
                                in0=e[:, c * HB : (c + 1) * HB],
                                in1=dec_b[:, c * HB : (c + 1) * HB],
                                op=mybir.AluOpType.mult,
                            )
                            dumpc = dump_pool.tile(
                                [P, HB], cdt, tag="dumpc", name="dumpc", bufs=2
                            )
                            nc.scalar.activation(
                                out=dumpc[:],
                                in_=prodc[:],
                                func=mybir.ActivationFunctionType.Copy,
                                accum_out=scp[c][:],
                            )
                        if c == 1:
                            with tc.tile_wait_until(t_base + 2 * t_step + 0.0007):
                                nc.gpsimd.tensor_add(scp[0][:], scp[0][:], scp[1][:])
                        if c == 2:
                            with tc.tile_wait_until(t_base + 3 * t_step + 0.0007):
                                nc.gpsimd.tensor_add(scp[0][:], scp[0][:], scp[2][:])
                    with tc.tile_wait_until(arrive[ti]):
                        scl = sc_pool.tile([P, 1], f32, tag="scl", name="scl")
                        prodl = prod_pool.tile(
                            [P, HB], cdt, tag="prodl", name="prodl", bufs=1
                        )
                        nc.vector.scalar_tensor_tensor(
                            out=prodl[:],
                            in0=e[:, 3 * HB :],
                            scalar=1.0,
                            in1=dec_b[:, 3 * HB :],
                            op0=mybir.AluOpType.mult,
                            op1=mybir.AluOpType.mult,
                            accum_out=scl[:],
                        )
                        nc.vector.tensor_add(sc_of[k][:], scp[0][:], scl[:])
                        ctx_matmuls(k, e, k == 0, True)
                    continue
                with tc.tile_wait_until(arrive[ti]):
                    if k in fused:
                        score_fused(e, sc_of[k])
                    else:
                        score_split(e, sc_of[k])
                    ctx_matmuls(k, e, k == 0, False)
                # PE p-state bursts: the tensor engine needs ~3us of
                # CONTINUOUS execution to reach 2.4 GHz (idle gaps drop it to
                # 1.2/0.65). Mid-stream that only shifts slack, but the tail
                # matmuls are latency-critical, so keep PE busy from u12's
                # matmuls through the end.
                if k == n_units - 4:
                    for j in range(12):
                        with tc.tile_wait_until(arrive[-4] + 0.0026 + 0.0002 * j):
                            nc.tensor.matmul(
                                warm_psum[:, :HB],
                                ones_row[:],
                                ones512[:],
                                start=True,
                                stop=True,
                            )
                elif k == n_units - 3:
                    for j in range(6):
                        with tc.tile_wait_until(arrive[-3] + 0.0026 + 0.0002 * j):
                            nc.tensor.matmul(
                                warm_psum[:, :HB],
                                ones_row[:],
                                ones512[:],
                                start=True,
                                stop=True,
                            )

            # softmax normalizer: [1,16] row of per-unit weight sums, folded
            # into the context output row (no separate norm DMA)
            nc.tensor.matmul(norm_psum[:], ones[:], w_all[:], start=True, stop=True)
            nc.vector.tensor_copy(out_sb[:, H : H + N_UNITS], norm_psum[:])
            for b in range(N_BANKS):
                eng = nc.vector.tensor_copy if b % 2 == 0 else nc.scalar.copy
                eng(out_sb[:, b * HB : (b + 1) * HB], ctx_psum[b][:])
            # two output DMAs on separate HWDGE queues, each fired as soon as
            # its pair of bank copies lands
            nc.sync.dma_start(out=ctx_d[:, 0 : 2 * HB], in_=out_sb[:, 0 : 2 * HB])
            nc.scalar.dma_start(
                out=ctx_d[:, 2 * HB :], in_=out_sb[:, 2 * HB :]
            )

    nc.compile()
    return nc


def _make_runner(nc):
    """Cached equivalent of bass2jax.run_bass_via_pjrt's multi-core path:
    build the sharded jitted executable once so warm calls skip re-tracing."""
    import jax
    import numpy as np
    from jax.experimental.shard_map import shard_map
    from jax.sharding import Mesh, PartitionSpec

    from concourse import bass2jax, mybir

    bass2jax.install_neuronx_cc_hook()
    assert nc.dbg_addr is None
    partition_name = nc.partition_id_tensor.name if nc.partition_id_tensor else None

    in_names, out_names, out_avals = [], [], []
    for alloc in nc.m.functions[0].allocations:
        if not isinstance(alloc, mybir.MemoryLocationSet):
            continue
        name = alloc.memorylocations[0].name
        if alloc.kind == "ExternalInput":
            if name != partition_name:
                in_names.append(name)
        elif alloc.kind == "ExternalOutput":
            out_names.append(name)
            out_avals.append(
                jax.core.ShapedArray(
                    tuple(alloc.tensor_shape), mybir.dt.np(alloc.dtype)
                )
            )
    n_params = len(in_names)
    all_in = list(in_names) + list(out_names)
    if partition_name is not None:
        all_in.append(partition_name)
    donate = tuple(range(n_params, n_params + len(out_names)))

    def _body(*args):
        operands = list(args)
        if partition_name is not None:
            operands.append(bass2jax.partition_id_tensor())
        return tuple(
            bass2jax._bass_exec_p.bind(
                *operands,
                out_avals=tuple(out_avals),
                in_names=tuple(all_in),
                out_names=tuple(out_names),
                lowering_input_output_aliases=(),
                sim_require_finite=True,
                sim_require_nnan=True,
                nc=nc,
            )
        )

    devices = jax.devices()[:N_CORES]
    mesh = Mesh(np.asarray(devices), ("core",))
    nio = n_params + len(out_names)
    sharded = jax.jit(
        shard_map(
            _body,
            mesh=mesh,
            in_specs=(PartitionSpec("core"),) * nio,
            out_specs=(PartitionSpec("core"),) * len(out_names),
            check_rep=False,
        ),
        donate_argnums=donate,
        keep_unused=True,
    )

    def run(in_maps):
        concat_in = [
            np.concatenate([m[name] for m in in_maps], axis=0) for name in in_names
        ]
        concat_zeros = [
            np.zeros((N_CORES * a.shape[0], *a.shape[1:]), a.dtype)
            for a in out_avals
        ]
        out_arrs = sharded(*concat_in, *concat_zeros)
        return [
            {
                name: np.asarray(out_arrs[i]).reshape(
                    N_CORES, *out_avals[i].shape
                )[c]
                for i, name in enumerate(out_names)
            }
            for c in range(N_CORES)
        ]

    return run


def _run(encoder_hiddens, decoder_hidden, trace=False, mm_dtype="f16", **kw):
    from concourse.bass_utils import run_bass_kernel_spmd

    key = f"nc_{mm_dtype}"
    if key not in _CACHE:
        _CACHE[key] = _build(mm_dtype)
    nc = _CACHE[key]

    enc = np.ascontiguousarray(encoder_hiddens, dtype=np.float32)
    dec = np.ascontiguousarray(decoder_hidden, dtype=np.float32)
    in_maps = [
        {
            "enc": enc[c * S_LOCAL : (c + 1) * S_LOCAL].reshape(-1),
            "dec": dec,
        }
        for c in range(N_CORES)
    ]
    if trace:
        res = run_bass_kernel_spmd(
            nc, in_maps, core_ids=list(range(N_CORES)), trace=True, **kw
        )
        results = res.results
    else:
        rkey = f"runner_{mm_dtype}"
        if rkey not in _CACHE:
            _CACHE[rkey] = _make_runner(nc)
        results = _CACHE[rkey](in_maps)
        res = None

    ctx = np.zeros((1, H), np.float64)
    z = 0.0
    for r in results:
        full = r["ctx"].astype(np.float64)
        ctx += full[:, :H]
        z += float(full[0, H:].sum())
    return (ctx / z).astype(np.float32), res


def kernel(encoder_hiddens, decoder_hidden):
    out, _ = _run(encoder_hiddens, decoder_hidden)
    return out

